# revision 1
# baseline (speedup 1.0000x reference)
"""Distributed Bass/Tile kernel for EnhancedDecoderAttention on 8 Trainium2 cores.

Module: q/k/v projections (+bias), rotate-halves RoPE on q/k, causal
masked softmax attention, output projection (+bias).
Shapes: x [4, 2048, 1024], 16 heads, head_dim 64.

Sharding: core c handles batch b = c//2 and head-half hh = c%2
(8 of 16 heads), i.e. column-sharded Wq/Wk/Wv, row-sharded Wo;
per-core partial outputs are summed pairwise on the host.

Design (measured 377 us/iter on 8xTRN2, vs 510-536 us for the prior
head-serial version):
  - attention processed per (head-PAIR, sq-chunk-of-512): the two heads of
    a pair live at partitions 0-63 / 64-127 of qh/kh, so their score
    matmuls are emitted back-to-back with tile_position (0,0)/(64,0) and
    run concurrently on disjoint PE row-groups (K=64 each).
  - scores psum [128, 1024] = A|B side by side (2 banks); one exp covers
    both heads for full tiles; exact-range exps on diagonal tiles.
  - ALL mask multiplies stay on DVE: routing head B's diagonal-mask mul
    through GpSimd costs ~216 us/iter in PE->ACT->Pool->PE latency.
  - chunk-major loop: out-projection for chunk c-1 overlaps attention of
    chunk c; only the last chunk's out-projection is a tail.
  - startup: q-projection and v-projection interleaved per xt chunk so PE
    keeps pace with the xt DMA stream; weights ride ahead on the SP ring.
  - head A of each pair evacuates straight from PSUM into anT (DVE);
    only head B needs the partition-shift DMA. Evacuation trails one pair
    behind so its reciprocal chain sits behind the next pair's mask muls
    in DVE's FIFO. (reciprocal_approx_fast is numerically broken on HW.)
"""

import numpy as np
import ml_dtypes
from contextlib import ExitStack

import concourse.bass as bass
import concourse.tile as tile
from concourse import bacc, mybir
from concourse.bass_utils import run_bass_kernel_spmd

BF = mybir.dt.bfloat16
F32 = mybir.dt.float32
AF = mybir.ActivationFunctionType

B, S, E, H, D = 4, 2048, 1024, 16, 64
NCORE = 8
HL = H // 2          # 8 local heads
NPAIR = HL // 2      # 4 local head pairs
EL = HL * D          # 512 local e_out
KE = E // 128        # 8 e_in tiles
NT = S // 128        # 16 sk tiles
NCH = S // 512       # 4 sq chunks
VS = 66              # v_s per-head stride (64 d + 1 ones + 1 pad)

_PROG_CACHE = {}


def _emit_body(nc, tc, ctx, aps, variant):
    causal = variant == "causal"
    masked = variant == "masked"

    per = ctx.enter_context(tc.tile_pool(name="per", bufs=1))
    qkp = ctx.enter_context(tc.tile_pool(name="qkp", bufs=2))
    expp = ctx.enter_context(tc.tile_pool(name="expp", bufs=4))
    outp = ctx.enter_context(tc.tile_pool(name="outp", bufs=4))
    smallp = ctx.enter_context(tc.tile_pool(name="smallp", bufs=3))
    pp = ctx.enter_context(tc.tile_pool(name="pp", bufs=2, space="PSUM"))
    pa = ctx.enter_context(tc.tile_pool(name="pa", bufs=2, space="PSUM"))
    pb = ctx.enter_context(tc.tile_pool(name="pb", bufs=2, space="PSUM"))
    if masked:
        mtp = ctx.enter_context(tc.tile_pool(name="mtp", bufs=4))

    # ---- persistent loads: xt chunks stream on the SP ring while the
    # q/v weights arrive in parallel on the Pool (SWDGE) ring ----
    xt_sb = per.tile([128, KE, S], BF)
    nc.sync.dma_start(
        xt_sb[:, :, 0:512],
        aps["xt"][:, 0:512].rearrange("(k p) s -> p k s", p=128))
    wq_sb = per.tile([128, KE, EL], BF)
    nc.sync.dma_start(wq_sb[:], aps["wq"].rearrange("(k p) n -> p k n", p=128))
    wv_sb = per.tile([128, KE, EL], BF)
    nc.sync.dma_start(wv_sb[:], aps["wv"].rearrange("(k p) n -> p k n", p=128))
    bv_sb = per.tile([128, EL], BF)
    nc.sync.dma_start(bv_sb[:], aps["bv_bc"][:])
    bq_sb = per.tile([128, 4], F32)
    nc.sync.dma_start(bq_sb[:], aps["bq"].rearrange("(m p) -> p m", p=128))
    bk_sb = per.tile([128, 4], F32)
    nc.sync.dma_start(bk_sb[:], aps["bk"].rearrange("(m p) -> p m", p=128))
    for c in range(1, NCH):
        nc.sync.dma_start(
            xt_sb[:, :, c * 512:(c + 1) * 512],
            aps["xt"][:, c * 512:(c + 1) * 512].rearrange("(k p) s -> p k s",
                                                          p=128))
    wk_sb = per.tile([128, KE, EL], BF)
    nc.sync.dma_start(wk_sb[:], aps["wk"].rearrange("(k p) n -> p k n", p=128))
    cos_sb = per.tile([128, S], BF)
    nc.sync.dma_start(cos_sb[:], aps["cos4"][:])
    sin_sb = per.tile([128, S], BF)
    nc.sync.dma_start(sin_sb[:], aps["sin4"][:])
    wo_sb = per.tile([128, 4, E], BF)
    nc.sync.dma_start(wo_sb[:], aps["wo"].rearrange("(k p) n -> p k n", p=128))
    if causal:
        dmask_sb = per.tile([128, 128], BF)
        nc.sync.dma_start(dmask_sb[:], aps["dmask"][:])

    qh_sb = [per.tile([128, S], BF, name=f"qh{i}") for i in range(NPAIR)]
    kh_sb = [per.tile([128, S], BF, name=f"kh{i}") for i in range(NPAIR)]
    vs_sb = [per.tile([128, HL, VS], BF, name=f"vs{i}") for i in range(NT)]
    anT_sb = [per.tile([128, S], BF, name=f"anT{i}") for i in range(NPAIR)]

    # ---- v projection (s-major), bias added during evacuation ----
    bv3 = bv_sb[:].rearrange("p (h d) -> p h d", d=D)

    def proj_v(trange):
        for t in trange:
            ps = pp.tile([128, 512], F32, tag="ps", name="ps_v")
            for ki in range(KE):
                nc.tensor.matmul(ps[:], xt_sb[:, ki, t * 128:(t + 1) * 128],
                                 wv_sb[:, ki, :], start=(ki == 0),
                                 stop=(ki == KE - 1))
            nc.gpsimd.memset(vs_sb[t][:, :, D:D + 1], 1.0)
            nc.vector.tensor_add(vs_sb[t][:, :, 0:D],
                                 ps[:].rearrange("p (h d) -> p h d", d=D), bv3)

    # ---- q/k projection + RoPE + repack for one head-group (2 pairs) ----
    qk_pre = {}

    def proj_qk_stream(g, which, crange):
        w_sb = wq_sb if which == "q" else wk_sb
        b_sb = bq_sb if which == "q" else bk_sb
        for part in range(2):  # 0 = r-half rows, 1 = i-half rows
            m = 2 * g + part
            if (g, which, part) not in qk_pre:
                qk_pre[(g, which, part)] = qkp.tile(
                    [128, S], BF, tag="pre", bufs=3,
                    name=f"pre{g}{which}{part}")
            prt = qk_pre[(g, which, part)]
            for c in crange:
                ps = pp.tile([128, 512], F32, tag="ps", name="ps_qk")
                for ki in range(KE):
                    nc.tensor.matmul(ps[:], w_sb[:, ki, m * 128:(m + 1) * 128],
                                     xt_sb[:, ki, c * 512:(c + 1) * 512],
                                     start=(ki == 0), stop=(ki == KE - 1))
                nc.vector.tensor_scalar_add(prt[:, c * 512:(c + 1) * 512],
                                            ps[:], b_sb[:, m:m + 1])

    def rope_repack(g, which):
        dsts = qh_sb if which == "q" else kh_sb
        pre = [qk_pre.pop((g, which, part)) for part in range(2)]
        # rot holds r-half in cols [0:S], i-half in cols [S:2S]
        rot = qkp.tile([128, 2 * S], BF, tag="rot", bufs=2,
                       name=f"rot{g}{which}")
        tmp = qkp.tile([128, S], BF, tag="tmp", bufs=2,
                       name=f"tmp{g}{which}")
        tmp2 = qkp.tile([128, S], BF, tag="tmp", bufs=2,
                        name=f"tmp2{g}{which}")
        nc.gpsimd.tensor_mul(tmp[:], pre[1][:], sin_sb[:])
        nc.vector.tensor_mul(rot[:, 0:S], pre[0][:], cos_sb[:])
        nc.vector.tensor_sub(rot[:, 0:S], rot[:, 0:S], tmp[:])
        nc.gpsimd.tensor_mul(tmp2[:], pre[0][:], sin_sb[:])
        nc.vector.tensor_mul(rot[:, S:2 * S], pre[1][:], cos_sb[:])
        nc.vector.tensor_add(rot[:, S:2 * S], rot[:, S:2 * S], tmp2[:])
        for hq in range(4):
            h = 4 * g + hq
            pair, off = h // 2, 64 * (h % 2)
            nc.sync.dma_start(dsts[pair][off:off + 32, :],
                              rot[hq * 32:(hq + 1) * 32, 0:S])
            nc.sync.dma_start(dsts[pair][off + 32:off + 64, :],
                              rot[hq * 32:(hq + 1) * 32, S:2 * S])

    # ---- attention for one (pair, sq-chunk-of-512) ----
    def attn_pair_chunk(pair, c):
        A, Bh = 2 * pair, 2 * pair + 1
        qh, kh = qh_sb[pair], kh_sb[pair]
        t_hi = 4 * (c + 1) if causal else NT
        c0, c1 = c * 512, (c + 1) * 512
        psbA = pb.tile([65, 512], F32, tag="psb", name=f"psbA{pair}{c}")
        psbB = pb.tile([65, 512], F32, tag="psb", name=f"psbB{pair}{c}")
        for t in range(t_hi):
            diag = causal and (t // 4 == c)
            lo = 128 * (t % 4) if diag else 0
            tl = t * 128
            ps = pa.tile([128, 1024], F32, tag="psa", name="ps_s")
            nc.tensor.matmul(ps[:, lo:512], kh[0:64, tl:tl + 128],
                             qh[0:64, c0 + lo:c1], start=True, stop=True)
            nc.tensor.matmul(ps[:, 512 + lo:1024], kh[64:128, tl:tl + 128],
                             qh[64:128, c0 + lo:c1], start=True, stop=True)
            ex = expp.tile([128, 1024], BF, tag="ex", name="ex")
            if lo:
                nc.scalar.activation(ex[:, lo:512], ps[:, lo:512],
                                     AF.Exp, scale=0.125)
                nc.scalar.activation(ex[:, 512 + lo:1024], ps[:, 512 + lo:1024],
                                     AF.Exp, scale=0.125)
            else:
                nc.scalar.activation(ex[:], ps[:], AF.Exp, scale=0.125)
            if diag:
                nc.vector.tensor_mul(ex[:, lo:lo + 128],
                                     ex[:, lo:lo + 128], dmask_sb[:])
                nc.vector.tensor_mul(ex[:, 512 + lo:512 + lo + 128],
                                     ex[:, 512 + lo:512 + lo + 128],
                                     dmask_sb[:])
            if masked:
                mt = mtp.tile([128, 512], BF, tag="mt", name="mt")
                nc.sync.dma_start(mt[:], aps["mt"][tl:tl + 128, c0:c1])
                nc.vector.tensor_mul(ex[:, 0:512], ex[:, 0:512], mt[:])
                nc.vector.tensor_mul(ex[:, 512:1024], ex[:, 512:1024], mt[:])
            last = (t == t_hi - 1)
            nc.tensor.matmul(psbA[:, lo:512], vs_sb[t][:, A, 0:65],
                             ex[:, lo:512], start=(t == 0), stop=last)
            nc.tensor.matmul(psbB[:, lo:512], vs_sb[t][:, Bh, 0:65],
                             ex[:, 512 + lo:1024], start=(t == 0), stop=last)
        return psbA, psbB

    # evacuation is DELAYED by one pair: its DVE ops (reciprocal chain) then
    # queue BEHIND the next pair's diagonal-mask muls in DVE's strict FIFO,
    # so they never stall the next pair's AV accumulation.
    def evac_pair(pair, c, psbA, psbB):
        c0, c1 = c * 512, (c + 1) * 512
        rBA = smallp.tile([1, 512], F32, tag="rB", name="rBA")
        nc.vector.reciprocal(rBA[:], psbA[64:65, :])
        repA = smallp.tile([64, 512], F32, tag="rep", name="repA")
        nc.gpsimd.partition_broadcast(repA[:], rBA[:])
        nc.vector.tensor_mul(anT_sb[pair][0:64, c0:c1], psbA[0:64, :], repA[:])
        rBB = smallp.tile([1, 512], F32, tag="rB", name="rBB")
        nc.vector.reciprocal(rBB[:], psbB[64:65, :])
        repB = smallp.tile([64, 512], F32, tag="rep", name="repB")
        nc.gpsimd.partition_broadcast(repB[:], rBB[:])
        anstB = smallp.tile([64, 512], BF, tag="anst", name="anstB")
        nc.vector.tensor_mul(anstB[:], psbB[0:64, :], repB[:])
        nc.sync.dma_start(anT_sb[pair][64:128, c0:c1], anstB[:])

    # ---- output projection for one sq-chunk: [E, 512] partial, transposed ----
    def outproj_chunk(c):
        c0, c1 = c * 512, (c + 1) * 512
        for et in range(KE):
            ps = pp.tile([128, 512], F32, tag="ps", name="ps_o")
            for pi in range(NPAIR):
                nc.tensor.matmul(ps[:], wo_sb[:, pi, et * 128:(et + 1) * 128],
                                 anT_sb[pi][:, c0:c1],
                                 start=(pi == 0), stop=(pi == NPAIR - 1))
            ot = outp.tile([128, 512], BF, tag="ot", name="ot")
            # ACT, not DVE: these copies run during attention chunks, where
            # DVE is the loaded engine on HW (mask muls + normalize chains)
            nc.scalar.copy(ot[:], ps[:])
            nc.sync.dma_start(
                aps["o"][et * 128:(et + 1) * 128, c0:c1], ot[:])

    # chunk-interleaved q/v start: each xt chunk feeds q-g0 + v as it lands
    for c in range(NCH):
        proj_qk_stream(0, "q", [c])
        proj_v(range(4 * c, 4 * c + 4))
    rope_repack(0, "q")
    proj_qk_stream(0, "k", range(NCH))
    rope_repack(0, "k")
    proj_qk_stream(1, "q", range(NCH))
    rope_repack(1, "q")
    proj_qk_stream(1, "k", range(NCH))
    rope_repack(1, "k")
    pending = None
    for c in range(NCH):
        for pair in range(NPAIR):
            psbs = attn_pair_chunk(pair, c)
            if pending is not None:
                evac_pair(*pending)
            pending = (pair, c, *psbs)
            if pair == 1 and c > 0:
                outproj_chunk(c - 1)
    evac_pair(*pending)
    outproj_chunk(NCH - 1)


def _build_program(variant, reps=1):
    key = (variant, reps)
    if key in _PROG_CACHE:
        return _PROG_CACHE[key]
    nc = bacc.Bacc("TRN2", target_bir_lowering=False, debug=False,
                   num_devices=NCORE)
    aps = {
        "xt": nc.dram_tensor("xt", [E, S], BF, kind="ExternalInput").ap(),
        "wq": nc.dram_tensor("wq", [E, EL], BF, kind="ExternalInput").ap(),
        "wk": nc.dram_tensor("wk", [E, EL], BF, kind="ExternalInput").ap(),
        "wv": nc.dram_tensor("wv", [E, EL], BF, kind="ExternalInput").ap(),
        "wo": nc.dram_tensor("wo", [EL, E], BF, kind="ExternalInput").ap(),
        "bq": nc.dram_tensor("bq", [EL], F32, kind="ExternalInput").ap(),
        "bk": nc.dram_tensor("bk", [EL], F32, kind="ExternalInput").ap(),
        "bv_bc": nc.dram_tensor("bv_bc", [128, EL], BF, kind="ExternalInput").ap(),
        "cos4": nc.dram_tensor("cos4", [128, S], BF, kind="ExternalInput").ap(),
        "sin4": nc.dram_tensor("sin4", [128, S], BF, kind="ExternalInput").ap(),
        "o": nc.dram_tensor("o", [E, S], BF, kind="ExternalOutput").ap(),
    }
    if variant == "causal":
        aps["dmask"] = nc.dram_tensor("dmask", [128, 128], BF,
                                      kind="ExternalInput").ap()
    if variant == "masked":
        aps["mt"] = nc.dram_tensor("mt", [S, S], BF, kind="ExternalInput").ap()

    with tile.TileContext(nc) as tc, ExitStack() as ctx:
        if reps > 1:
            with tc.For_i(0, reps, 1):
                _emit_body(nc, tc, ctx, aps, variant)
        else:
            _emit_body(nc, tc, ctx, aps, variant)
    nc.compile()
    _PROG_CACHE[key] = nc
    return nc


def _rope_tables():
    inv_freq = 1.0 / (10000.0 ** (np.arange(0, D, 2, dtype=np.float64) / D))
    pos = np.arange(S, dtype=np.float64)
    freqs = pos[:, None] * inv_freq[None, :]          # [S, 32]
    cos = np.cos(freqs).T.astype(np.float32)          # [32, S]
    sin = np.sin(freqs).T.astype(np.float32)
    cos4 = np.tile(cos, (4, 1)).astype(ml_dtypes.bfloat16)  # [128, S]
    sin4 = np.tile(sin, (4, 1)).astype(ml_dtypes.bfloat16)
    return cos4, sin4


def _qk_perm():
    # projection output column order: [r-rows heads 0-3 | i-rows heads 0-3 |
    #                                  r-rows heads 4-7 | i-rows heads 4-7]
    perm = []
    for g in range(2):
        for part in range(2):
            for h in range(4 * g, 4 * g + 4):
                for dd in range(32):
                    perm.append(h * D + part * 32 + dd)
    return np.array(perm)


def _prep_inputs(x, mask, Wq, bq, Wk, bk, Wv, bv, Wo, bo):
    x = np.asarray(x, dtype=np.float32)
    mask = np.asarray(mask).astype(bool)
    to_np = lambda a: np.asarray(a, dtype=np.float32)
    Wq, bq, Wk, bk = to_np(Wq), to_np(bq), to_np(Wk), to_np(bk)
    Wv, bv, Wo, bo = to_np(Wv), to_np(bv), to_np(Wo), to_np(bo)

    if mask.all():
        variant = "dense"
    elif np.array_equal(mask, np.tril(np.ones((S, S), dtype=bool))):
        variant = "causal"
    else:
        variant = "masked"

    cos4, sin4 = _rope_tables()
    perm = _qk_perm()
    bf = ml_dtypes.bfloat16

    in_maps = []
    common = {}
    if variant == "causal":
        jj = np.arange(128)
        common["dmask"] = (jj[None, :] >= jj[:, None]).astype(bf)
    if variant == "masked":
        common["mt"] = mask.T.astype(bf)
    for c in range(NCORE):
        b, hh = c // 2, c % 2
        sl = slice(hh * EL, (hh + 1) * EL)
        m = {
            "xt": np.ascontiguousarray(x[b].T).astype(bf),
            "wq": Wq[:, sl][:, perm].astype(bf),
            "wk": Wk[:, sl][:, perm].astype(bf),
            "wv": Wv[:, sl].astype(bf),
            "wo": Wo[sl, :].astype(bf),
            "bq": np.ascontiguousarray(bq[sl][perm]),
            "bk": np.ascontiguousarray(bk[sl][perm]),
            "bv_bc": np.tile(bv[sl][None, :], (128, 1)).astype(bf),
            "cos4": cos4,
            "sin4": sin4,
        }
        m.update(common)
        in_maps.append(m)
    return variant, in_maps, bo


def kernel(x, mask, Wq, bq, Wk, bk, Wv, bv, Wo, bo):
    variant, in_maps, bo_np = _prep_inputs(x, mask, Wq, bq, Wk, bk, Wv, bv,
                                           Wo, bo)
    nc = _build_program(variant)
    res = None
    last_err = None
    for _attempt in range(3):
        try:
            res = run_bass_kernel_spmd(nc, in_maps, list(range(NCORE)))
            break
        except Exception as e:  # sporadic NRT device flakes: retry
            last_err = e
            import time as _time
            _time.sleep(3)
    if res is None:
        raise last_err
    out = np.empty((B, S, E), dtype=np.float32)
    for b in range(B):
        acc = (res.results[2 * b]["o"].astype(np.float32)
               + res.results[2 * b + 1]["o"].astype(np.float32))
        out[b] = acc.T + bo_np[None, :]
    return out

